# revision 3
# baseline (speedup 1.0000x reference)
"""Trainium2 Bass kernel for the masked-logsumexp multi-branch loss.

Problem: logit [524288, 128] f32, target [524288, 128] int32 (multi-hot 0/1).
Per row r (is_null = target[r,0]):
  branch1 (is_null): lse(all) - x0
  branch2: (n_pos*lse({0} u pos) - sum_pos_logit)/max(n_pos,1) + lse(neg u {0}) - x0
Output: scalar mean over all rows.

Data-parallel over 8 NeuronCores (65536 rows each), rows on SBUF partitions.
Logits are N(0,1) so exp() cannot overflow; the three masked logsumexps share
unshifted per-row sums: S_E = sum exp(x), S_ME = sum M*exp(x),
S_MX = sum M*x, S_M = sum M, plus the class-0 column extracts.

Per [128, 16*128] super-tile:
  ScalarE : mask int32->bf16, exp(x)->bf16, column extracts
  GPSIMD  : M*x product (bf16 mask x f32 logits -> bf16), replacing both
            the ScalarE x->bf16 copy and one DVE product
  VectorE : one bf16 2x-mode product (M*E); one shared fold chain
            (four 2x halving adds over [mask|M*E|M*x|E]) + short 1x reduce
Per-row combine runs on [128, 128] f32 stat tiles interleaved with the loop;
per-row losses go back to HBM; the final mean is taken on host.

Measured on trn2 (8 cores): ~269 us HW exec, output rel err ~9e-6.
Note: this container's walrus accepts one sync-wait per instruction, so
_split_sync_waits() rewrites the Tile-scheduled BIR accordingly.
"""
import numpy as np

import concourse.bass as bass
import concourse.tile as tile
from concourse import mybir
from concourse.bass_utils import run_bass_kernel_spmd

B = 524288
C = 128
NCORES = 8
RPC = B // NCORES  # rows per core = 65536
P = 128  # partitions
NB = 16  # class-blocks per super-tile -> [128, NB*128] tiles
ST = RPC // (P * NB)  # super-tiles per core = 32
NSTATS = ST * NB  # stat columns per core = 512
NACC = 8  # blocks per super-tile whose exp uses the fused ScalarE accumulator

FP32 = mybir.dt.float32
BF16 = mybir.dt.bfloat16
I32 = mybir.dt.int32
ALU = mybir.AluOpType
AF = mybir.ActivationFunctionType


def _build_kernel(tc: tile.TileContext, lo, logit, target):
    nc = tc.nc
    # row = (s*128 + p)*NB + n ; per (s, p): NB*C contiguous f32 = 8KB DMA lines
    Xd = logit.rearrange("(s p n) c -> s p (n c)", p=P, n=NB)
    Md = target.rearrange("(s p n) c -> s p (n c)", p=P, n=NB)
    LOd = lo.rearrange("(s p n) -> p s n", p=P, n=NB)

    import contextlib

    with contextlib.ExitStack() as ctx:
        stats = ctx.enter_context(tc.tile_pool(name="stats", bufs=1))
        work = ctx.enter_context(tc.tile_pool(name="work", bufs=3))
        dmap = ctx.enter_context(tc.tile_pool(name="dmap", bufs=4))
        fpool = ctx.enter_context(tc.tile_pool(name="fpool", bufs=2))
        small = ctx.enter_context(tc.tile_pool(name="small", bufs=2))

        # Persistent per-core stat arrays: S_ALL[q] for q in [M, ME, MX, E]
        S_ALL = stats.tile([P, 4, ST, NB], FP32)
        X0 = stats.tile([P, ST, NB], FP32)
        IS0 = stats.tile([P, ST, NB], FP32)

        lot = small.tile([P, NSTATS], FP32, tag="lot")

        # ---- combine (already emitted interleaved above) ----
        NCH = 4
        W = NSTATS // NCH

        def flat(t):
            return t.rearrange("p a b -> p (a b)")

        def combine_chunk(ch):
            sl = slice(ch * W, (ch + 1) * W)
            sM = flat(S_ALL[:, 0])[:, sl]
            sME = flat(S_ALL[:, 1])[:, sl]
            sMX = flat(S_ALL[:, 2])[:, sl]
            sE = flat(S_ALL[:, 3])[:, sl]
            x0 = flat(X0)[:, sl]
            m0 = flat(IS0)[:, sl]

            E0 = small.tile([P, W], FP32, tag="c0")
            t_a = small.tile([P, W], FP32, tag="c1")
            t_b = small.tile([P, W], FP32, tag="c2")
            lse_all = small.tile([P, W], FP32, tag="c3")
            lse_pos = small.tile([P, W], FP32, tag="c4")
            lse_neg = small.tile([P, W], FP32, tag="c5")
            npos = small.tile([P, W], FP32, tag="c6")
            rinv = small.tile([P, W], FP32, tag="c7")
            acc = small.tile([P, W], FP32, tag="c8")

            # E0 = exp(x0)
            nc.scalar.activation(out=E0, in_=x0, func=AF.Exp)
            # t_a = S_ME - M0*E0   (= sum_{c>=1} M*E)
            nc.vector.tensor_mul(t_b, m0, E0)
            nc.vector.tensor_sub(t_a, sME, t_b)
            # lse_pos = log(E0 + t_a)
            nc.vector.tensor_add(t_b, t_a, E0)
            nc.scalar.activation(out=lse_pos, in_=t_b, func=AF.Ln)
            # lse_neg = log(max(S_E - t_a, E0))
            nc.vector.tensor_sub(t_b, sE, t_a)
            nc.vector.tensor_tensor(out=t_b, in0=t_b, in1=E0, op=ALU.max)
            nc.scalar.activation(out=lse_neg, in_=t_b, func=AF.Ln)
            # lse_all = log(S_E)
            nc.scalar.activation(out=lse_all, in_=sE, func=AF.Ln)
            # n_pos = S_M - M0 ; rinv = 1/max(n_pos, 1)
            nc.vector.tensor_sub(npos, sM, m0)
            nc.vector.tensor_scalar_max(t_b, npos, 1.0)
            # 1/n via ScalarE: exp(-ln n) (same ACT table set as Exp/Ln)
            nc.scalar.activation(out=rinv, in_=t_b, func=AF.Ln)
            nc.scalar.activation(out=rinv, in_=rinv, func=AF.Exp, scale=-1.0)
            # acc = (n_pos*lse_pos - (S_MX - M0*x0)) * rinv + lse_neg
            nc.vector.tensor_mul(t_b, m0, x0)
            nc.vector.tensor_sub(t_b, sMX, t_b)
            nc.vector.tensor_mul(t_a, npos, lse_pos)
            nc.vector.tensor_sub(t_a, t_a, t_b)
            nc.vector.tensor_mul(t_a, t_a, rinv)
            nc.vector.tensor_add(acc, t_a, lse_neg)
            # lo = M0*(lse_all - acc) + acc - x0
            nc.vector.tensor_sub(t_a, lse_all, acc)
            nc.vector.tensor_mul(t_a, t_a, m0)
            nc.vector.tensor_add(t_a, t_a, acc)
            nc.vector.tensor_sub(lot[:, sl], t_a, x0)


        for s in range(ST):
            xt = dmap.tile([P, NB, C], FP32, tag="xt")
            mt = dmap.tile([P, NB, C], I32, tag="mt")
            nc.sync.dma_start(out=xt, in_=Xd[s].rearrange("p (n c) -> p n c", c=C))
            nc.sync.dma_start(out=mt, in_=Md[s].rearrange("p (n c) -> p n c", c=C))

            # Q holds the four reduction operands: [mbf, pme, pmx, et]
            Q = work.tile([P, 4, NB, C], BF16, tag="Q")
            mbf = Q[:, 0]
            pme = Q[:, 1]
            pmx = Q[:, 2]
            et = Q[:, 3]

            # ScalarE: int32 -> bf16 mask convert ; exp
            nc.scalar.copy(out=mbf, in_=mt)
            nc.scalar.activation(out=et, in_=xt, func=AF.Exp)
            # column extracts (class 0) on ScalarE
            nc.scalar.copy(out=X0[:, s, :], in_=xt[:, :, 0])
            nc.scalar.copy(out=IS0[:, s, :], in_=mt[:, :, 0])

            # GPSIMD product: M*x reads the f32 logits directly
            nc.gpsimd.tensor_mul(pmx, mbf, xt)
            # VectorE product (bf16 2x)
            nc.vector.tensor_mul(pme, et, mbf)

            # single fold chain over all four quantities (bf16 2x adds),
            # then a short 1x reduce over the last 16
            f1 = fpool.tile([P, 4, NB, C // 2], BF16, tag="f1")
            f2 = fpool.tile([P, 4, NB, C // 4], BF16, tag="f2")
            f3 = fpool.tile([P, 4, NB, C // 8], BF16, tag="f3")
            f4 = fpool.tile([P, 4, NB, C // 16], BF16, tag="f4")
            f5 = fpool.tile([P, 4, NB, C // 32], BF16, tag="f5")
            nc.vector.tensor_add(f1, Q[:, :, :, 0 : C // 2], Q[:, :, :, C // 2 : C])
            nc.vector.tensor_add(
                f2, f1[:, :, :, 0 : C // 4], f1[:, :, :, C // 4 : C // 2]
            )
            nc.vector.tensor_add(
                f3, f2[:, :, :, 0 : C // 8], f2[:, :, :, C // 8 : C // 4]
            )
            nc.vector.tensor_add(
                f4, f3[:, :, :, 0 : C // 16], f3[:, :, :, C // 16 : C // 8]
            )
            nc.vector.tensor_add(
                f5, f4[:, :, :, 0 : C // 32], f4[:, :, :, C // 32 : C // 16]
            )
            nc.vector.reduce_sum(
                out=S_ALL[:, :, s, :], in_=f5, axis=mybir.AxisListType.X
            )
            if ST % NCH == 0 and (s + 1) % (ST // NCH) == 0:
                combine_chunk((s + 1) // (ST // NCH) - 1)

        if ST % NCH != 0:
            for ch in range(NCH):
                combine_chunk(ch)

        nc.sync.dma_start(out=LOd, in_=lot.rearrange("p (s n) -> p s n", n=NB))


def _split_sync_waits(nc):
    """The container's walrus accepts at most ONE sync-wait command per
    instruction (the TPB EVENTS struct has a single wait slot). Tile emits
    instructions with N waits; rewrite each so the extra waits ride on
    same-engine NoOps inserted immediately before (engine program order makes
    this semantically identical)."""
    for f in nc.m.functions:
        for blk in f.blocks:
            insts = blk.instructions
            out = []
            changed = False
            for inst in insts:
                si = inst.sync_info
                waits = list(si.on_wait) if (si is not None and si.on_wait) else []
                if len(waits) > 1:
                    changed = True
                    for k, w in enumerate(waits[:-1]):
                        nop = mybir.InstNoOp(name=f"{inst.name}-w{k}", ins=[], outs=[])
                        nop.engine = inst.engine
                        nop.sync_info = mybir.SyncInfo(on_wait=[w], on_update=[])
                        out.append(nop)
                    inst.sync_info = mybir.SyncInfo(
                        on_wait=[waits[-1]],
                        on_update=list(si.on_update) if si.on_update else [],
                    )
                out.append(inst)
            if changed:
                blk.instructions = out


_NC_CACHE = None
SPLIT_WAITS = True


def _get_nc():
    global _NC_CACHE
    if _NC_CACHE is None:
        nc = bass.Bass()
        logit = nc.declare_dram_parameter("logit", [RPC, C], FP32, isOutput=False)
        target = nc.declare_dram_parameter("target", [RPC, C], I32, isOutput=False)
        lo = nc.declare_dram_parameter("lo", [RPC], FP32, isOutput=True)
        with tile.TileContext(nc) as tc:
            _build_kernel(tc, lo, logit, target)
        if SPLIT_WAITS:
            _split_sync_waits(nc)
        _NC_CACHE = nc
    return _NC_CACHE


def kernel(**inputs) -> np.ndarray:
    logit = np.ascontiguousarray(np.asarray(inputs["logit"], dtype=np.float32))
    target = np.ascontiguousarray(np.asarray(inputs["target"], dtype=np.int32))
    assert logit.shape == (B, C) and target.shape == (B, C)

    nc = _get_nc()
    in_maps = [
        {
            "logit": logit[i * RPC : (i + 1) * RPC],
            "target": target[i * RPC : (i + 1) * RPC],
        }
        for i in range(NCORES)
    ]
    res = run_bass_kernel_spmd(nc, in_maps, core_ids=list(range(NCORES)))
    lo = np.concatenate([r["lo"].reshape(-1) for r in res.results])
    return np.array(np.mean(lo, dtype=np.float64), dtype=np.float32)



# revision 5
# speedup vs baseline: 1.1068x; 1.1068x over previous
"""Trainium2 Bass kernel for the masked-logsumexp multi-branch loss.

Problem: logit [524288, 128] f32, target [524288, 128] int32 (multi-hot 0/1).
Per row r (is_null = target[r,0]):
  branch1 (is_null): lse(all) - x0
  branch2: (n_pos*lse({0} u pos) - sum_pos_logit)/max(n_pos,1) + lse(neg u {0}) - x0
Output: scalar mean over all rows.

Data-parallel over 8 NeuronCores (65536 rows each), rows on SBUF partitions.
Logits are N(0,1) so exp() cannot overflow; the three masked logsumexps share
unshifted per-row sums: S_E = sum exp(x), S_ME = sum M*exp(x),
S_MX = sum M*x, S_M = sum M, plus the class-0 column extracts.

Per [128, 32*128] super-tile (W5 = [mbf|pme|pmx|et|xbf] stacked in one tile):
  ScalarE : mask int32->bf16, exp(x)->bf16, x->bf16, column extracts
  VectorE : ONE fused dual product (broadcast mask AP x [et|xbf], bf16 2x);
            one shared IN-PLACE fold chain (five 2x halving adds over
            [mask|M*E|M*x|E] written back into the same tile) + 1x reduce
  (GPSIMD/TensorE unused: Pool's shared SBUF port steals ~0.6ns/ns of DVE
   throughput - measured; PE can only contract the partition axis.)
Per-row combine runs on [128, 128] f32 stat tiles interleaved with the loop;
per-row losses go back to HBM; the final mean is taken on host.

Note: this container's walrus accepts one sync-wait per instruction, so
_split_sync_waits() rewrites the Tile-scheduled BIR accordingly.
"""
import numpy as np

import concourse.bass as bass
import concourse.tile as tile
from concourse import mybir
from concourse.bass_utils import run_bass_kernel_spmd

B = 524288
C = 128
NCORES = 8
RPC = B // NCORES  # rows per core = 65536
P = 128  # partitions
NB = 32  # class-blocks per super-tile -> [128, NB*128] tiles
ST = RPC // (P * NB)  # super-tiles per core = 16
NSTATS = ST * NB  # stat columns per core = 512

FP32 = mybir.dt.float32
BF16 = mybir.dt.bfloat16
I32 = mybir.dt.int32
ALU = mybir.AluOpType
AF = mybir.ActivationFunctionType


def _build_kernel(tc: tile.TileContext, lo, logit, target):
    nc = tc.nc
    # row = (s*128 + p)*NB + n ; per (s, p): NB*C contiguous f32 = 16KB DMA lines
    Xd = logit.rearrange("(s p n) c -> s p (n c)", p=P, n=NB)
    Md = target.rearrange("(s p n) c -> s p (n c)", p=P, n=NB)
    LOd = lo.rearrange("(s p n) -> p s n", p=P, n=NB)

    import contextlib

    with contextlib.ExitStack() as ctx:
        stats = ctx.enter_context(tc.tile_pool(name="stats", bufs=1))
        work = ctx.enter_context(tc.tile_pool(name="work", bufs=2))
        dmap = ctx.enter_context(tc.tile_pool(name="dmap", bufs=3))
        small = ctx.enter_context(tc.tile_pool(name="small", bufs=2))

        # Persistent per-core stat arrays: S_ALL[q] for q in [M, ME, MX, E]
        S_ALL = stats.tile([P, 4, ST, NB], FP32)
        X0 = stats.tile([P, ST, NB], FP32)
        IS0 = stats.tile([P, ST, NB], FP32)

        lot = small.tile([P, NSTATS], FP32, tag="lot")

        # ---- combine (emitted interleaved below) ----
        NCH = 4
        W = NSTATS // NCH

        def flat(t):
            return t.rearrange("p a b -> p (a b)")

        def combine_chunk(ch):
            sl = slice(ch * W, (ch + 1) * W)
            sM = flat(S_ALL[:, 0])[:, sl]
            sME = flat(S_ALL[:, 1])[:, sl]
            sMX = flat(S_ALL[:, 2])[:, sl]
            sE = flat(S_ALL[:, 3])[:, sl]
            x0 = flat(X0)[:, sl]
            m0 = flat(IS0)[:, sl]

            E0 = small.tile([P, W], FP32, tag="c0")
            t_a = small.tile([P, W], FP32, tag="c1")
            t_b = small.tile([P, W], FP32, tag="c2")
            lse_all = small.tile([P, W], FP32, tag="c3")
            lse_pos = small.tile([P, W], FP32, tag="c4")
            lse_neg = small.tile([P, W], FP32, tag="c5")
            npos = small.tile([P, W], FP32, tag="c6")
            rinv = small.tile([P, W], FP32, tag="c7")
            acc = small.tile([P, W], FP32, tag="c8")

            # E0 = exp(x0)
            nc.scalar.activation(out=E0, in_=x0, func=AF.Exp)
            # t_a = S_ME - M0*E0   (= sum_{c>=1} M*E)
            nc.vector.tensor_mul(t_b, m0, E0)
            nc.vector.tensor_sub(t_a, sME, t_b)
            # lse_pos = log(E0 + t_a)
            nc.vector.tensor_add(t_b, t_a, E0)
            nc.scalar.activation(out=lse_pos, in_=t_b, func=AF.Ln)
            # lse_neg = log(max(S_E - t_a, E0))
            nc.vector.tensor_sub(t_b, sE, t_a)
            nc.vector.tensor_tensor(out=t_b, in0=t_b, in1=E0, op=ALU.max)
            nc.scalar.activation(out=lse_neg, in_=t_b, func=AF.Ln)
            # lse_all = log(S_E)
            nc.scalar.activation(out=lse_all, in_=sE, func=AF.Ln)
            # n_pos = S_M - M0 ; rinv = 1/max(n_pos, 1)
            nc.vector.tensor_sub(npos, sM, m0)
            nc.vector.tensor_scalar_max(t_b, npos, 1.0)
            # 1/n via ScalarE: exp(-ln n) (same ACT table set as Exp/Ln)
            nc.scalar.activation(out=rinv, in_=t_b, func=AF.Ln)
            nc.scalar.activation(out=rinv, in_=rinv, func=AF.Exp, scale=-1.0)
            # acc = (n_pos*lse_pos - (S_MX - M0*x0)) * rinv + lse_neg
            nc.vector.tensor_mul(t_b, m0, x0)
            nc.vector.tensor_sub(t_b, sMX, t_b)
            nc.vector.tensor_mul(t_a, npos, lse_pos)
            nc.vector.tensor_sub(t_a, t_a, t_b)
            nc.vector.tensor_mul(t_a, t_a, rinv)
            nc.vector.tensor_add(acc, t_a, lse_neg)
            # lo = M0*(lse_all - acc) + acc - x0
            nc.vector.tensor_sub(t_a, lse_all, acc)
            nc.vector.tensor_mul(t_a, t_a, m0)
            nc.vector.tensor_add(t_a, t_a, acc)
            nc.vector.tensor_sub(lot[:, sl], t_a, x0)

        for s in range(ST):
            xt = dmap.tile([P, NB, C], FP32, tag="xt")
            mt = dmap.tile([P, NB, C], I32, tag="mt")
            nc.sync.dma_start(out=xt, in_=Xd[s].rearrange("p (n c) -> p n c", c=C))
            nc.sync.dma_start(out=mt, in_=Md[s].rearrange("p (n c) -> p n c", c=C))

            # W5 stacks [mbf, pme, pmx, et, xbf]; Q = first four get folded
            W5 = work.tile([P, 5, NB, C], BF16, tag="W5")
            mbf = W5[:, 0]
            et = W5[:, 3]
            xbf = W5[:, 4]
            Q = W5[:, 0:4]

            # ScalarE: int32 -> bf16 mask convert ; exp ; bf16 logits
            nc.scalar.copy(out=mbf, in_=mt)
            nc.scalar.activation(out=et, in_=xt, func=AF.Exp)
            nc.scalar.copy(out=xbf, in_=xt)
            # column extracts (class 0) on ScalarE
            nc.scalar.copy(out=X0[:, s, :], in_=xt[:, :, 0])
            nc.scalar.copy(out=IS0[:, s, :], in_=mt[:, :, 0])

            # ONE fused dual product: [pme|pmx] = bcast(mbf) * [et|xbf]
            nc.vector.tensor_tensor(
                out=W5[:, 1:3],
                in0=mbf[:, None].broadcast_to([P, 2, NB, C]),
                in1=W5[:, 3:5],
                op=ALU.mult,
            )

            # In-place fold chain over the four streams (bf16 2x halving adds)
            for w in (64, 32, 16, 8, 4):
                nc.vector.tensor_add(
                    Q[:, :, :, 0:w], Q[:, :, :, 0:w], Q[:, :, :, w : 2 * w]
                )
            nc.vector.reduce_sum(
                out=S_ALL[:, :, s, :], in_=Q[:, :, :, 0:4], axis=mybir.AxisListType.X
            )
            if ST % NCH == 0 and (s + 1) % (ST // NCH) == 0:
                combine_chunk((s + 1) // (ST // NCH) - 1)

        if ST % NCH != 0:
            for ch in range(NCH):
                combine_chunk(ch)

        nc.sync.dma_start(out=LOd, in_=lot.rearrange("p (s n) -> p s n", n=NB))


def _split_sync_waits(nc):
    """The container's walrus accepts at most ONE sync-wait command per
    instruction (the TPB EVENTS struct has a single wait slot). Tile emits
    instructions with N waits; rewrite each so the extra waits ride on
    same-engine NoOps inserted immediately before (engine program order makes
    this semantically identical)."""
    for f in nc.m.functions:
        for blk in f.blocks:
            insts = blk.instructions
            out = []
            changed = False
            for inst in insts:
                si = inst.sync_info
                waits = list(si.on_wait) if (si is not None and si.on_wait) else []
                if len(waits) > 1:
                    changed = True
                    for k, w in enumerate(waits[:-1]):
                        nop = mybir.InstNoOp(name=f"{inst.name}-w{k}", ins=[], outs=[])
                        nop.engine = inst.engine
                        nop.sync_info = mybir.SyncInfo(on_wait=[w], on_update=[])
                        out.append(nop)
                    inst.sync_info = mybir.SyncInfo(
                        on_wait=[waits[-1]],
                        on_update=list(si.on_update) if si.on_update else [],
                    )
                out.append(inst)
            if changed:
                blk.instructions = out


_NC_CACHE = None
SPLIT_WAITS = True


def _get_nc():
    global _NC_CACHE
    if _NC_CACHE is None:
        nc = bass.Bass()
        logit = nc.declare_dram_parameter("logit", [RPC, C], FP32, isOutput=False)
        target = nc.declare_dram_parameter("target", [RPC, C], I32, isOutput=False)
        lo = nc.declare_dram_parameter("lo", [RPC], FP32, isOutput=True)
        with tile.TileContext(nc) as tc:
            _build_kernel(tc, lo, logit, target)
        if SPLIT_WAITS:
            _split_sync_waits(nc)
        _NC_CACHE = nc
    return _NC_CACHE


def kernel(**inputs) -> np.ndarray:
    logit = np.ascontiguousarray(np.asarray(inputs["logit"], dtype=np.float32))
    target = np.ascontiguousarray(np.asarray(inputs["target"], dtype=np.int32))
    assert logit.shape == (B, C) and target.shape == (B, C)

    nc = _get_nc()
    in_maps = [
        {
            "logit": logit[i * RPC : (i + 1) * RPC],
            "target": target[i * RPC : (i + 1) * RPC],
        }
        for i in range(NCORES)
    ]
    res = run_bass_kernel_spmd(nc, in_maps, core_ids=list(range(NCORES)))
    lo = np.concatenate([r["lo"].reshape(-1) for r in res.results])
    return np.array(np.mean(lo, dtype=np.float64), dtype=np.float32)


# revision 6
# speedup vs baseline: 1.3306x; 1.2022x over previous
"""Trainium2 Bass kernel for the masked-logsumexp multi-branch loss.

Problem: logit [524288, 128] f32, target [524288, 128] int32 (multi-hot 0/1).
Per row r (is_null = target[r,0]):
  branch1 (is_null): lse(all) - x0
  branch2: (n_pos*lse({0} u pos) - sum_pos_logit)/max(n_pos,1) + lse(neg u {0}) - x0
Output: scalar mean over all rows.

Data-parallel over 8 NeuronCores (65536 rows each), rows on SBUF partitions.
Logits are N(0,1) so exp() cannot overflow; the three masked logsumexps share
unshifted per-row sums: S_E = sum exp(x), S_ME = sum M*exp(x),
S_MX = sum M*x, S_M = sum M, plus the class-0 column extracts.

Per [128, 32*128] super-tile (W5 = [mbf|pme|pmx|et|xbf] stacked in one tile):
  ScalarE : mask int32->bf16, exp(x)->bf16, x->bf16, column extracts
  VectorE : ONE fused dual product (broadcast mask AP x [et|xbf], bf16 2x);
            one shared IN-PLACE fold chain (five 2x halving adds over
            [mask|M*E|M*x|E] written back into the same tile) + 1x reduce
  (GPSIMD/TensorE unused: Pool's shared SBUF port steals ~0.6ns/ns of DVE
   throughput - measured; PE can only contract the partition axis.)
Per-row combine runs on [128, 128] f32 stat tiles interleaved with the loop;
per-row losses go back to HBM; the final mean is taken on host.

Note: this container's walrus accepts one sync-wait per instruction, so
_split_sync_waits() rewrites the Tile-scheduled BIR accordingly.
"""
import numpy as np

import concourse.bass as bass
import concourse.tile as tile
from concourse import mybir
from concourse.bass_utils import run_bass_kernel_spmd

B = 524288
C = 128
NCORES = 8
RPC = B // NCORES  # rows per core = 65536
P = 128  # partitions
NB = 32  # class-blocks per super-tile -> [128, NB*128] tiles
ST = RPC // (P * NB)  # super-tiles per core = 16
NSTATS = ST * NB  # stat columns per core = 512

FP32 = mybir.dt.float32
BF16 = mybir.dt.bfloat16
I32 = mybir.dt.int32
ALU = mybir.AluOpType
AF = mybir.ActivationFunctionType


def _build_kernel(tc: tile.TileContext, lo, logit, target):
    nc = tc.nc
    # row = (s*128 + p)*NB + n ; per (s, p): NB*C contiguous f32 = 16KB DMA lines
    Xd = logit.rearrange("(s p n) c -> s p (n c)", p=P, n=NB)
    Md = target.rearrange("(s p n) c -> s p (n c)", p=P, n=NB)
    LOd = lo.rearrange("(s p n) -> p s n", p=P, n=NB)

    import contextlib

    with contextlib.ExitStack() as ctx:
        stats = ctx.enter_context(tc.tile_pool(name="stats", bufs=1))
        work = ctx.enter_context(tc.tile_pool(name="work", bufs=3))
        dmap = ctx.enter_context(tc.tile_pool(name="dmap", bufs=2))
        small = ctx.enter_context(tc.tile_pool(name="small", bufs=1))

        # Persistent per-core stat arrays: S_ALL[q] for q in [M, ME, MX, E]
        S_ALL = stats.tile([P, 4, ST, NB], FP32)
        X0 = stats.tile([P, ST, NB], FP32)
        IS0 = stats.tile([P, ST, NB], FP32)

        lot = small.tile([P, NSTATS], FP32, tag="lot")

        # ---- combine (emitted interleaved below) ----
        NCH = 4
        W = NSTATS // NCH

        def flat(t):
            return t.rearrange("p a b -> p (a b)")

        def combine_chunk(ch):
            sl = slice(ch * W, (ch + 1) * W)
            sM = flat(S_ALL[:, 0])[:, sl]
            sME = flat(S_ALL[:, 1])[:, sl]
            sMX = flat(S_ALL[:, 2])[:, sl]
            sE = flat(S_ALL[:, 3])[:, sl]
            x0 = flat(X0)[:, sl]
            m0 = flat(IS0)[:, sl]

            E0 = small.tile([P, W], FP32, tag="c0")
            t_a = small.tile([P, W], FP32, tag="c1")
            t_b = small.tile([P, W], FP32, tag="c2")
            lse_all = small.tile([P, W], FP32, tag="c3")
            lse_pos = small.tile([P, W], FP32, tag="c4")
            lse_neg = small.tile([P, W], FP32, tag="c5")
            npos = small.tile([P, W], FP32, tag="c6")
            rinv = small.tile([P, W], FP32, tag="c7")
            acc = small.tile([P, W], FP32, tag="c8")

            # E0 = exp(x0)
            nc.scalar.activation(out=E0, in_=x0, func=AF.Exp)
            # t_a = S_ME - M0*E0   (= sum_{c>=1} M*E)
            nc.vector.tensor_mul(t_b, m0, E0)
            nc.vector.tensor_sub(t_a, sME, t_b)
            # lse_pos = log(E0 + t_a)
            nc.vector.tensor_add(t_b, t_a, E0)
            nc.scalar.activation(out=lse_pos, in_=t_b, func=AF.Ln)
            # lse_neg = log(max(S_E - t_a, E0))
            nc.vector.tensor_sub(t_b, sE, t_a)
            nc.vector.tensor_tensor(out=t_b, in0=t_b, in1=E0, op=ALU.max)
            nc.scalar.activation(out=lse_neg, in_=t_b, func=AF.Ln)
            # lse_all = log(S_E)
            nc.scalar.activation(out=lse_all, in_=sE, func=AF.Ln)
            # n_pos = S_M - M0 ; rinv = 1/max(n_pos, 1)
            nc.vector.tensor_sub(npos, sM, m0)
            nc.vector.tensor_scalar_max(t_b, npos, 1.0)
            # 1/n via ScalarE: exp(-ln n) (same ACT table set as Exp/Ln)
            nc.scalar.activation(out=rinv, in_=t_b, func=AF.Ln)
            nc.scalar.activation(out=rinv, in_=rinv, func=AF.Exp, scale=-1.0)
            # acc = (n_pos*lse_pos - (S_MX - M0*x0)) * rinv + lse_neg
            nc.vector.tensor_mul(t_b, m0, x0)
            nc.vector.tensor_sub(t_b, sMX, t_b)
            nc.vector.tensor_mul(t_a, npos, lse_pos)
            nc.vector.tensor_sub(t_a, t_a, t_b)
            nc.vector.tensor_mul(t_a, t_a, rinv)
            nc.vector.tensor_add(acc, t_a, lse_neg)
            # lo = M0*(lse_all - acc) + acc - x0
            nc.vector.tensor_sub(t_a, lse_all, acc)
            nc.vector.tensor_mul(t_a, t_a, m0)
            nc.vector.tensor_add(t_a, t_a, acc)
            nc.vector.tensor_sub(lot[:, sl], t_a, x0)

        for s in range(ST):
            xt = dmap.tile([P, NB, C], FP32, tag="xt")
            mt = dmap.tile([P, NB, C], I32, tag="mt")
            nc.sync.dma_start(out=xt, in_=Xd[s].rearrange("p (n c) -> p n c", c=C))
            nc.sync.dma_start(out=mt, in_=Md[s].rearrange("p (n c) -> p n c", c=C))

            # W5 stacks [mbf, pme, pmx, et, xbf]; Q = first four get folded
            W5 = work.tile([P, 5, NB, C], BF16, tag="W5")
            mbf = W5[:, 0]
            et = W5[:, 3]
            xbf = W5[:, 4]
            Q = W5[:, 0:4]

            # ScalarE: int32 -> bf16 mask convert ; exp ; bf16 logits
            nc.scalar.copy(out=mbf, in_=mt)
            nc.scalar.activation(out=et, in_=xt, func=AF.Exp)
            nc.scalar.copy(out=xbf, in_=xt)
            # column extracts (class 0) on ScalarE
            nc.scalar.copy(out=X0[:, s, :], in_=xt[:, :, 0])
            nc.scalar.copy(out=IS0[:, s, :], in_=mt[:, :, 0])

            # ONE fused dual product: [pme|pmx] = bcast(mbf) * [et|xbf]
            nc.vector.tensor_tensor(
                out=W5[:, 1:3],
                in0=mbf[:, None].broadcast_to([P, 2, NB, C]),
                in1=W5[:, 3:5],
                op=ALU.mult,
            )

            # In-place fold chain over the four streams (bf16 2x halving adds)
            for w in (64, 32, 16, 8, 4):
                nc.vector.tensor_add(
                    Q[:, :, :, 0:w], Q[:, :, :, 0:w], Q[:, :, :, w : 2 * w]
                )
            nc.vector.reduce_sum(
                out=S_ALL[:, :, s, :], in_=Q[:, :, :, 0:4], axis=mybir.AxisListType.X
            )
            if ST % NCH == 0 and (s + 1) % (ST // NCH) == 0:
                combine_chunk((s + 1) // (ST // NCH) - 1)

        if ST % NCH != 0:
            for ch in range(NCH):
                combine_chunk(ch)

        nc.sync.dma_start(out=LOd, in_=lot.rearrange("p (s n) -> p s n", n=NB))


def _split_sync_waits(nc):
    """The container's walrus accepts at most ONE sync-wait command per
    instruction (the TPB EVENTS struct has a single wait slot). Tile emits
    instructions with N waits; rewrite each so the extra waits ride on
    same-engine NoOps inserted immediately before (engine program order makes
    this semantically identical)."""
    for f in nc.m.functions:
        for blk in f.blocks:
            insts = blk.instructions
            out = []
            changed = False
            for inst in insts:
                si = inst.sync_info
                waits = list(si.on_wait) if (si is not None and si.on_wait) else []
                if len(waits) > 1:
                    changed = True
                    for k, w in enumerate(waits[:-1]):
                        nop = mybir.InstNoOp(name=f"{inst.name}-w{k}", ins=[], outs=[])
                        nop.engine = inst.engine
                        nop.sync_info = mybir.SyncInfo(on_wait=[w], on_update=[])
                        out.append(nop)
                    inst.sync_info = mybir.SyncInfo(
                        on_wait=[waits[-1]],
                        on_update=list(si.on_update) if si.on_update else [],
                    )
                out.append(inst)
            if changed:
                blk.instructions = out


_NC_CACHE = None
SPLIT_WAITS = True


def _get_nc():
    global _NC_CACHE
    if _NC_CACHE is None:
        nc = bass.Bass()
        logit = nc.declare_dram_parameter("logit", [RPC, C], FP32, isOutput=False)
        target = nc.declare_dram_parameter("target", [RPC, C], I32, isOutput=False)
        lo = nc.declare_dram_parameter("lo", [RPC], FP32, isOutput=True)
        with tile.TileContext(nc) as tc:
            _build_kernel(tc, lo, logit, target)
        if SPLIT_WAITS:
            _split_sync_waits(nc)
        _NC_CACHE = nc
    return _NC_CACHE


def kernel(**inputs) -> np.ndarray:
    logit = np.ascontiguousarray(np.asarray(inputs["logit"], dtype=np.float32))
    target = np.ascontiguousarray(np.asarray(inputs["target"], dtype=np.int32))
    assert logit.shape == (B, C) and target.shape == (B, C)

    nc = _get_nc()
    in_maps = [
        {
            "logit": logit[i * RPC : (i + 1) * RPC],
            "target": target[i * RPC : (i + 1) * RPC],
        }
        for i in range(NCORES)
    ]
    res = run_bass_kernel_spmd(nc, in_maps, core_ids=list(range(NCORES)))
    lo = np.concatenate([r["lo"].reshape(-1) for r in res.results])
    return np.array(np.mean(lo, dtype=np.float64), dtype=np.float32)
